# revision 18
# baseline (speedup 1.0000x reference)
# Trainium2 Bass kernel for nn_CustomStyleLoss (segment-mean + MSE reduction).
#
# loss = sum_rows mean_chunks( (mean_chunk(input) - mean_chunk(style))^2 )
# with rows = 16*512 = 8192, each row = 50*50 = 2500 elems = 25 chunks of 100.
#
# Data-parallel over the row axis: core i gets rows [i*1024, (i+1)*1024).
# Raw Bass (no Tile framework). Per core: 9 pieces per tensor (7 full
# [128 x 2500] tiles + the last tile split 2000+500 cols) cycling through
# 6 SBUF slots. Input pieces stream on the SP HWDGE ring, style pieces on
# the ACT ring; the 16 SDMA engines drain both rings at the ~384 GB/s
# HBM-per-core share (~53.4us for the 20.5MB shard). Big 1.28MB DMAs and
# the [128, 6, 2500] slot layout are load-bearing: half-tile streaming
# measured ~20% slower, and an 8-slot layout made the DVE scan 20% slower
# (operand-stream bank conflicts at the shifted relative offset).
#
# Compute per piece: the DVE runs the fused subtract+prefix-scan
# (tensor_tensor_scan, the fastest single-pass fp32 primitive at ~2.15
# ns/elem), one drain (the scan does not flush before a dependent strided
# read), and one strided difference for the chunk sums. The
# square+accumulate runs on the otherwise-idle ACT engine (activation
# Square with accum_out into a per-piece partials column) for all pieces
# but the last, trimming two DVE ops per piece off the serial chain; cs
# has one slot per piece so there is no DVE/ACT buffer hazard. Each DMA
# pair incs one shared semaphore (single DVE wait per piece), the final
# 500-col piece is squared+reduced on the DVE itself (no ACT wake on the
# critical path), and the ACT engine ships the result. The 2000+500 tail
# split leaves only ~1.7us of DVE work after the last byte lands instead
# of a full 6us tile. Loss scale is applied on the host.

import sys

if "/opt/trn_rl_repo" not in sys.path:
    sys.path.insert(0, "/opt/trn_rl_repo")

import numpy as np

import concourse.bass as bass
from concourse import mybir
from concourse.bass_utils import run_bass_kernel_spmd

N_CORES = 8
N_ROWS = 8192
K = 2500
CHUNK = 100
P = 128
CPL = K // CHUNK
ROWS_PER_CORE = N_ROWS // N_CORES
N_TILES = ROWS_PER_CORE // P
N_BUFS = 6
SPLIT = 2000
PIECES = [(t, 0, K) for t in range(N_TILES - 1)] + [
    (N_TILES - 1, 0, SPLIT),
    (N_TILES - 1, SPLIT, K),
]
N_PIECES = len(PIECES)              # 9
_SLOT = [t % N_BUFS for (t, _, _) in PIECES]
SCALE = 1.0 / (CHUNK * np.sqrt(CPL))
SCALE2 = float(SCALE * SCALE)

_CACHED_NC = None


def _prev_user(i):
    t = PIECES[i][0]
    prev_t = t - N_BUFS
    if prev_t < 0:
        return None
    for j, (tj, _, _) in enumerate(PIECES):
        if tj == prev_t:
            return j
    return None


def _build_nc():
    nc = bass.Bass(
        "TRN2", target_bir_lowering=False, debug=False, num_devices=N_CORES
    )
    x = nc.dram_tensor(
        "input", [ROWS_PER_CORE, K], mybir.dt.float32, kind="ExternalInput"
    ).ap()
    s = nc.dram_tensor(
        "style", [ROWS_PER_CORE, K], mybir.dt.float32, kind="ExternalInput"
    ).ap()
    o = nc.dram_tensor(
        "out", [P, N_PIECES], mybir.dt.float32, kind="ExternalOutput"
    ).ap()

    from contextlib import ExitStack

    with ExitStack() as ctx:
        xt = ctx.enter_context(
            nc.sbuf_tensor("xt", [P, N_BUFS, K], mybir.dt.float32)
        )
        st = ctx.enter_context(
            nc.sbuf_tensor("st", [P, N_BUFS, K], mybir.dt.float32)
        )
        sc = ctx.enter_context(
            nc.sbuf_tensor("sc", [P, K + 1], mybir.dt.float32)
        )
        cs = ctx.enter_context(
            nc.sbuf_tensor("cs", [P, N_PIECES, CPL], mybir.dt.float32)
        )
        sq = ctx.enter_context(nc.sbuf_tensor("sq", [P, CPL], mybir.dt.float32))
        sqv = ctx.enter_context(nc.sbuf_tensor("sqv", [P, CPL], mybir.dt.float32))
        partials = ctx.enter_context(
            nc.sbuf_tensor("partials", [P, N_PIECES], mybir.dt.float32)
        )
        s_pair = [
            ctx.enter_context(nc.semaphore(f"s_pair{i}")) for i in range(N_PIECES)
        ]
        s_d = ctx.enter_context(nc.semaphore("s_d"))
        s_cs = ctx.enter_context(nc.semaphore("s_cs"))
        s_out = ctx.enter_context(nc.semaphore("s_out"))
        block = ctx.enter_context(nc.Block(no_gpsimd_drain=True))

        def src(t_ap, piece):
            t, c0, c1 = piece
            return t_ap[t * P : (t + 1) * P, c0:c1]

        def dst(t_sb, i):
            t, c0, c1 = PIECES[i]
            return t_sb[:, _SLOT[i], c0:c1]

        @block.sync
        def _(sync):
            for i, piece in enumerate(PIECES):
                p = _prev_user(i)
                if p is not None:
                    sync.wait_ge(s_d, p + 1)
                sync.dma_start(out=dst(xt, i), in_=src(x, piece)).then_inc(
                    s_pair[i], 16
                )

        @block.scalar
        def _(scalar):
            for i, piece in enumerate(PIECES):
                p = _prev_user(i)
                if p is not None:
                    scalar.wait_ge(s_d, p + 1)
                scalar.dma_start(out=dst(st, i), in_=src(s, piece)).then_inc(
                    s_pair[i], 16
                )
            for i, piece in enumerate(PIECES[:-1]):
                nch = (piece[2] - piece[1]) // CHUNK
                scalar.wait_ge(s_d, i + 1)
                nc.scalar.activation(
                    out=sq[:, 0:nch],
                    in_=cs[:, i, 0:nch],
                    func=mybir.ActivationFunctionType.Square,
                    accum_out=partials[:, i : i + 1],
                ).then_inc(s_cs, 1)
            # Scalar ships the result once the DVE's last-piece square lands:
            # the final 500-col piece is squared on the DVE (skips one
            # cross-engine hop). No wait on the out receipt (postamble gives
            # the 4.6KB write ample time to land).
            scalar.wait_ge(s_cs, N_PIECES)
            scalar.drain()
            scalar.dma_start(out=o, in_=partials[:]).then_inc(s_out, 16)

        @block.vector
        def _(vector):
            nc.vector.memset(sc[:, 0:1], 0.0)
            for i, piece in enumerate(PIECES):
                w = piece[2] - piece[1]
                nch = w // CHUNK
                vector.wait_ge(s_pair[i], 32)
                nc.vector.tensor_tensor_scan(
                    out=sc[:, 1 : w + 1],
                    data0=dst(xt, i),
                    data1=dst(st, i),
                    initial=0.0,
                    op0=mybir.AluOpType.add,
                    op1=mybir.AluOpType.subtract,
                )
                vector.drain()
                nc.vector.tensor_sub(
                    cs[:, i, 0:nch],
                    sc[:, CHUNK : w + 1 : CHUNK],
                    sc[:, 0:w:CHUNK],
                ).then_inc(s_d, 1)
            # Last piece's square+accumulate stays on the DVE: partials[:, -1]
            # lands without waiting for an ACT wake at the very end. Drains
            # around the strided ops (same non-flush hazard as the scan).
            last = N_PIECES - 1
            nlast = (PIECES[last][2] - PIECES[last][1]) // CHUNK
            vector.drain()
            nc.vector.tensor_mul(
                sqv[:, 0:nlast], cs[:, last, 0:nlast], cs[:, last, 0:nlast]
            )
            vector.drain()
            nc.vector.tensor_reduce(
                out=partials[:, last : last + 1],
                in_=sqv[:, 0:nlast],
                axis=mybir.AxisListType.X,
                op=mybir.AluOpType.add,
            ).then_inc(s_cs, 1)

    return nc


def _get_nc():
    global _CACHED_NC
    if _CACHED_NC is None:
        _CACHED_NC = _build_nc()
    return _CACHED_NC


def run_sharded(input, style, **run_kwargs):
    nc = _get_nc()
    xi = np.ascontiguousarray(np.asarray(input, dtype=np.float32)).reshape(
        N_ROWS, K
    )
    xs = np.ascontiguousarray(np.asarray(style, dtype=np.float32)).reshape(
        N_ROWS, K
    )
    in_maps = [
        {
            "input": xi[i * ROWS_PER_CORE : (i + 1) * ROWS_PER_CORE],
            "style": xs[i * ROWS_PER_CORE : (i + 1) * ROWS_PER_CORE],
        }
        for i in range(N_CORES)
    ]
    res = run_bass_kernel_spmd(nc, in_maps, list(range(N_CORES)), **run_kwargs)
    total = np.float64(0.0)
    for r in res.results:
        total += r["out"].astype(np.float64).sum()
    return np.array(total * SCALE2, dtype=np.float32), res


def kernel(input, style):
    loss, _ = run_sharded(input, style)
    return loss
